# revision 29
# baseline (speedup 1.0000x reference)
"""GCN message-passing kernel for 8 Trainium2 NeuronCores.

Strategy: shard NODES (destinations) across the 8 cores; each core owns all
edges whose dst lands in its node slice, so no cross-core reduction is needed.

The gather feature[src] dominates (memory regime). It runs as a few large
`dma_gather` ops per core: the host converts the feature table to bf16 (256B
rows), builds int16 index vectors (wrapped into 16 partitions and replicated
across the 8 Q7 cores), and splits edges into src<32768 / src>=32768 groups so
int16 indexing works against two base offsets of the table. Windows of 128
destination nodes are grouped (GROUP windows per gather pair) and each group's
edge tiles are fetched by one lo + one hi dma_gather into SBUF.

Per 128-edge tile, a bf16 one-hot (built on DVE in the 4x perf mode) against
the window's node iota scatter-weights the messages, and a PE matmul
accumulates the window's [feature, node] sums in PSUM: the segment-sum is
sum_e m[e, f] * onehot[e, n] with weight 1/deg[dst] (mean aggregation) folded
into the one-hot. Padding slots carry weight 0, so every index is valid and
all cores run the identical SPMD schedule.

Epilogue per window: PSUM->SBUF copy on ACT (bf16), linear via PE, ReLU+bias
on ACT, and a DMA of the [dout, node] window into a transposed DRAM output
(the host untransposes). Isolated nodes keep their input feature via host-
added self-edges with weight 1.
"""
import os
import sys
sys.path.insert(0, "/opt/trn_rl_repo")
os.environ.setdefault("NEURON_RT_RESET_CORES", "1")

import numpy as np
import ml_dtypes
import concourse.bass as bass
import concourse.bacc as bacc
import concourse.mybir as mybir
import concourse.tile as tile
from concourse.bass_utils import run_bass_kernel_spmd

P = 128
N_NODES = 50000
D = 128
N_CORES = 8
WIN = 128                      # nodes per window (= PSUM tile free size)
WINS_PER_CORE = 49             # 49 * 128 = 6272 node slots per core
NPC = WINS_PER_CORE * WIN      # 6272 node slots (core 7 has ghost tail)
SPLIT = 32768                  # int16 index range per dma_gather base
GROUP = 7                      # windows per message buffer
OP_TILES = 8                   # tiles per dma_gather op (1024-idx ucode cap)


def _pack_idx16(vals16, n_slots):
    """int16 slot-ordered vals -> [128, n_slots//16] wrapped + Q7-replicated."""
    S = n_slots // 16
    flat = np.zeros(S * 16, np.int16)
    flat[: len(vals16)] = vals16
    block = flat.reshape(S, 16).T
    return np.ascontiguousarray(np.tile(block, (8, 1)))


def _host_schedule(feature, W, b, src, dst, n_nodes=N_NODES, n_cores=N_CORES,
                   wins_per_core=WINS_PER_CORE):
    """Shard + sort + pad edges; build per-core input tensors."""
    npc = wins_per_core * WIN
    n_wins = wins_per_core
    deg = np.bincount(dst, minlength=n_nodes).astype(np.int64)
    recip = 1.0 / np.maximum(deg, 1).astype(np.float32)

    iso = np.where(deg == 0)[0].astype(np.int64)
    if iso.size:
        src = np.concatenate([src, iso])
        dst = np.concatenate([dst, iso])
    E = src.shape[0]

    hi = (src >= SPLIT).astype(np.int64)   # int16 range group

    # Balance windows across (core, slot): global window g = dst >> 7 is
    # assigned a (core, slot) pair so that the 8 windows sharing a slot have
    # similar edge counts — the SPMD-shared tile schedule T[slot] is a max
    # over cores, so sorted-octet assignment minimizes ceil padding.
    n_gwins = n_cores * n_wins
    gwin = dst >> 7                        # global window of each edge
    gcnt = np.zeros(n_gwins, np.int64)
    np.add.at(gcnt, gwin, 1)
    rank = np.argsort(-gcnt, kind="stable")            # windows by load, desc
    core_of_gwin = np.empty(n_gwins, np.int64)
    slot_of_gwin = np.empty(n_gwins, np.int64)
    core_of_gwin[rank] = np.tile(np.arange(n_cores), n_wins)
    slot_of_gwin[rank] = np.repeat(np.arange(n_wins), n_cores)

    core = core_of_gwin[gwin]              # owning core per edge
    win = slot_of_gwin[gwin]               # window slot within core
    dloc_in_win = (dst & 127).astype(np.int64)

    # counts per (core, slot, lo/hi) -> shared tile schedule
    cnt = np.zeros((n_cores, n_wins, 2), dtype=np.int64)
    np.add.at(cnt, (core, win, hi), 1)
    T_lh = -(-cnt.max(axis=0) // P)                    # [n_wins, 2] tiles
    empty = T_lh.sum(axis=1) == 0
    T_lh[empty, 0] = 1                                 # keep >=1 tile per window
    T_LO, T_HI = T_lh[:, 0], T_lh[:, 1]

    groups = [list(range(g, min(g + GROUP, n_wins)))
              for g in range(0, n_wins, GROUP)]

    # global tile-column order: per group, all lo tiles (window order), then
    # all hi tiles (window order). Each run is chopped into <=OP_TILES-tile
    # dma_gather ops (1024-idx ucode cap).
    col_lo = np.zeros(n_wins, np.int64)    # first global col of window's lo run
    col_hi = np.zeros(n_wins, np.int64)
    gather_ops = []                        # (num_idxs, is_hi, group, local_col0)
    colc = 0
    for gi, grp in enumerate(groups):
        lo_g = int(sum(T_LO[w] for w in grp))
        hi_g = int(sum(T_HI[w] for w in grp))
        c = colc
        for w in grp:
            col_lo[w] = c
            c += T_LO[w]
        for w in grp:
            col_hi[w] = c
            c += T_HI[w]
        # equalize op sizes within each run: tail ops below ~512 idx run far
        # below line rate, so split a run of R tiles into ceil(R/OP_TILES)
        # near-equal ops instead of 8,8,...,remainder
        lc = 0
        for run_tiles, is_hi in ((lo_g, 0), (hi_g, 1)):
            if run_tiles == 0:
                continue
            n_ops = -(-run_tiles // OP_TILES)
            base, extra = divmod(run_tiles, n_ops)
            for oi_ in range(n_ops):
                nt_op = base + (1 if oi_ < extra else 0)
                gather_ops.append((nt_op * P, is_hi, gi, lc))
                lc += nt_op
        colc += lo_g + hi_g
    NT = int(colc)                         # total tile columns per core

    # slot of each edge: col_base(win, hi)*P + position within its bucket
    order = np.lexsort((src, hi, win, core))
    s_src = src[order]
    s_core = core[order]
    s_win = win[order]
    s_hi = hi[order]
    s_dloc = dloc_in_win[order].astype(np.float32)
    s_w = recip[dst[order]]

    bucket = (s_core * n_wins + s_win) * 2 + s_hi
    bcnt = np.bincount(bucket, minlength=n_cores * n_wins * 2)
    bstarts = np.concatenate([[0], np.cumsum(bcnt)])
    pos = np.arange(E) - bstarts[bucket]
    col_base = np.where(s_hi == 1, col_hi[s_win], col_lo[s_win])
    slot = col_base * P + pos

    srcs = np.zeros((n_cores, NT * P), dtype=np.int64)
    dlocs = np.zeros((n_cores, NT * P), dtype=np.float32)
    ws = np.zeros((n_cores, NT * P), dtype=np.float32)
    hi_slot = np.zeros(NT * P, dtype=bool)             # schedule-wide
    for w in range(n_wins):
        hi_slot[col_hi[w] * P:(col_hi[w] + T_HI[w]) * P] = True
    srcs[s_core, slot] = s_src
    # padding slots in hi tiles must index the hi base
    srcs[:, hi_slot] = np.maximum(srcs[:, hi_slot], SPLIT)
    dlocs[s_core, slot] = s_dloc
    ws[s_core, slot] = s_w

    # int16 index stream per gather op (op-local slot order), concatenated
    idx16_cols = []
    op_off16 = []
    off = 0
    scol = 0
    for num_idxs, is_hi, _gi, _lc in gather_ops:
        op_off16.append(off)
        if num_idxs == 0:
            continue
        sl = slice(scol * P, scol * P + num_idxs)
        vals = srcs[:, sl] - (SPLIT if is_hi else 0)
        assert vals.min() >= 0 and vals.max() < SPLIT
        idx16_cols.append(
            np.stack([_pack_idx16(vals[c].astype(np.int16), num_idxs)
                      for c in range(n_cores)]))
        scol += num_idxs // P
        off += num_idxs // 16
    gidx = np.concatenate(idx16_cols, axis=2)          # [n_cores, 128, off]

    gwin_of = np.empty((n_cores, n_wins), np.int64)
    gwin_of[core_of_gwin, slot_of_gwin] = np.arange(n_gwins)

    # map global tile column -> (op index, column within op)
    col_op = np.empty(NT, np.int64)
    col_in_op = np.empty(NT, np.int64)
    c0 = 0
    for oi, (n_idx, _h, _g, _lc) in enumerate(gather_ops):
        nt_op = n_idx // P
        col_op[c0:c0 + nt_op] = oi
        col_in_op[c0:c0 + nt_op] = np.arange(nt_op)
        c0 += nt_op
    assert c0 == NT

    sched = {
        "T_LO": T_LO, "T_HI": T_HI, "groups": groups,
        "col_lo": col_lo, "col_hi": col_hi,
        "gather_ops": gather_ops, "op_off16": op_off16,
        "NT": NT, "S16": int(off), "gwin_of": gwin_of,
        "col_op": col_op, "col_in_op": col_in_op,
    }

    feature16 = np.ascontiguousarray(
        np.asarray(feature, np.float32).astype(ml_dtypes.bfloat16))
    Wf = np.ascontiguousarray(
        np.asarray(W, np.float32).astype(ml_dtypes.bfloat16))
    bf = np.ascontiguousarray(b, dtype=np.float32).reshape(P, 1)
    in_maps = []
    for c in range(n_cores):
        in_maps.append({
            "feat": feature16,
            "gidx": np.ascontiguousarray(gidx[c]),
            "dstloc": np.ascontiguousarray(dlocs[c].reshape(NT, P).T),
            "wcol": np.ascontiguousarray(ws[c].reshape(NT, P).T),
            "Wmat": Wf,
            "bias": bf,
        })
    return in_maps, sched


def _build(sched, reps=1, scratch=16384, mbufs=26, n_nodes=N_NODES,
           wins_per_core=WINS_PER_CORE, n_cores=N_CORES):
    npc = wins_per_core * WIN
    NT, S16 = sched["NT"], sched["S16"]
    T_LO, T_HI = sched["T_LO"], sched["T_HI"]
    groups = sched["groups"]
    gather_ops = sched["gather_ops"]
    op_off16 = sched["op_off16"]
    f32 = mybir.dt.float32
    bf16 = mybir.dt.bfloat16

    nc = bacc.Bacc("TRN2", debug=False, num_devices=n_cores,
                   dynamic_dma_scratch_size=scratch, num_swdge_queues=1)
    feat = nc.dram_tensor("feat", [n_nodes, D], bf16, kind="ExternalInput")
    gidx_d = nc.dram_tensor("gidx", [P, S16], mybir.dt.int16, kind="ExternalInput")
    dstloc = nc.dram_tensor("dstloc", [P, NT], f32, kind="ExternalInput")
    wcol = nc.dram_tensor("wcol", [P, NT], f32, kind="ExternalInput")
    Wmat = nc.dram_tensor("Wmat", [D, D], bf16, kind="ExternalInput")
    bias = nc.dram_tensor("bias", [P, 1], f32, kind="ExternalInput")
    # output is [dout, node] (transposed, bf16); host untransposes + upcasts
    out = nc.dram_tensor("out", [D, npc], bf16, kind="ExternalOutput")

    n_lo_rows = min(SPLIT, n_nodes)

    with tile.TileContext(nc) as tc:
        with (
            tc.tile_pool(name="const", bufs=1) as cpool,
            tc.tile_pool(name="msgs", bufs=mbufs) as mpool,
            tc.tile_pool(name="oh", bufs=12) as ohpool,
            tc.tile_pool(name="hwin", bufs=3) as hpool,
            tc.tile_pool(name="outw", bufs=3) as opool,
            tc.tile_pool(name="ph1", bufs=3, space="PSUM") as p1pool,
            tc.tile_pool(name="ph2", bufs=2, space="PSUM") as p2pool,
        ):
            gidx_t = cpool.tile([P, S16], mybir.dt.int16)
            nc.sync.dma_start(gidx_t[:], gidx_d[:])
            dst_t = cpool.tile([P, NT], f32)
            nc.sync.dma_start(dst_t[:], dstloc[:])
            w_t = cpool.tile([P, NT], f32)
            nc.sync.dma_start(w_t[:], wcol[:])
            Wt = cpool.tile([D, D], bf16)
            nc.sync.dma_start(Wt[:], Wmat[:])
            b_t = cpool.tile([P, 1], f32)
            nc.sync.dma_start(b_t[:], bias[:])

            iota_i = cpool.tile([P, WIN], mybir.dt.int32)
            nc.gpsimd.iota(iota_i[:], pattern=[[1, WIN]], base=0, channel_multiplier=0)
            iota_f = cpool.tile([P, WIN], bf16)
            nc.vector.tensor_copy(iota_f[:], iota_i[:])

            col_op = sched["col_op"]
            col_in_op = sched["col_in_op"]
            for rep in range(reps):
              gcol = 0                       # global tile column
              op_tiles = {}
              for gi, grp in enumerate(groups):
                  for oi, (n_idx, is_hi, op_gi, lc) in enumerate(gather_ops):
                      if op_gi != gi or n_idx == 0:
                          continue
                      o16 = op_off16[oi]
                      nt_op = n_idx // P
                      m_op = mpool.tile([P, nt_op * D], bf16)
                      op_tiles[oi] = m_op
                      nc.gpsimd.dma_gather(
                          out_ap=m_op[:].rearrange("p (c e) -> p c e", e=D),
                          in_ap=(feat[SPLIT:n_nodes] if is_hi
                                 else feat[0:n_lo_rows]),
                          idxs_ap=gidx_t[:, o16:o16 + n_idx // 16],
                          num_idxs=n_idx, num_idxs_reg=n_idx,
                          elem_size=D, queue_num=0)

                  lo_g = int(sum(T_LO[w] for w in grp))
                  lo_c, hi_c = 0, lo_g
                  for w in grp:
                      cols = (list(range(lo_c, lo_c + int(T_LO[w]))) +
                              list(range(hi_c, hi_c + int(T_HI[w]))))
                      lo_c += int(T_LO[w])
                      hi_c += int(T_HI[w])
                      ph1 = p1pool.tile([D, WIN], f32, space="PSUM")
                      for j, c in enumerate(cols):
                          gc = gcol + c
                          oh = ohpool.tile([P, WIN], bf16)
                          nc.vector.tensor_scalar(
                              out=oh[:], in0=iota_f[:],
                              scalar1=dst_t[:, gc:gc + 1], scalar2=w_t[:, gc:gc + 1],
                              op0=mybir.AluOpType.is_equal, op1=mybir.AluOpType.mult,
                          )
                          mo = op_tiles[col_op[gc]]
                          cl = int(col_in_op[gc])
                          # ph1[f, n] += sum_e m[e, f] * oh[e, n]
                          nc.tensor.matmul(out=ph1[:], lhsT=mo[:, cl * D:(cl + 1) * D],
                                           rhs=oh[:], start=(j == 0),
                                           stop=(j == len(cols) - 1))

                      # hT window -> SBUF (ACT keeps DVE free); bf16 so the
                      # second matmul streams at 1 cycle/row
                      hT = hpool.tile([D, WIN], bf16)
                      nc.scalar.activation(hT[:], ph1[:],
                                           mybir.ActivationFunctionType.Copy,
                                           bias=0.0, scale=1.0)
                      ph2 = p2pool.tile([D, WIN], f32, space="PSUM")
                      nc.tensor.matmul(out=ph2[:], lhsT=Wt[:], rhs=hT[:],
                                       start=True, stop=True)
                      s2 = opool.tile([D, WIN], bf16)
                      nc.scalar.activation(s2[:], ph2[:],
                                           mybir.ActivationFunctionType.Relu,
                                           bias=b_t[:, 0:1], scale=1.0)
                      nc.sync.dma_start(out[:, w * WIN:(w + 1) * WIN], s2[:])
                  gcol += lo_g + int(sum(T_HI[w] for w in grp))
    nc.compile()
    return nc


_CACHE = {}


def _assemble(outs, sched, n_nodes=N_NODES, n_cores=N_CORES,
              wins_per_core=WINS_PER_CORE):
    """outs[c] = per-core [D, npc] device output; returns full [n_nodes, D]."""
    gwin_of = sched["gwin_of"]
    out = np.empty((n_nodes, D), dtype=np.float32)
    for c in range(n_cores):
        oc = outs[c]
        for w in range(wins_per_core):
            gw = gwin_of[c, w]
            lo = gw * WIN
            hi = min(lo + WIN, n_nodes)
            if lo >= n_nodes:
                continue
            out[lo:hi] = oc[:, w * WIN:w * WIN + (hi - lo)].T.astype(np.float32)
    return out


def kernel(feature, W, b, src, dst):
    feature = np.asarray(feature, dtype=np.float32)
    W = np.asarray(W, dtype=np.float32)
    b = np.asarray(b, dtype=np.float32)
    src = np.asarray(src, dtype=np.int64)
    dst = np.asarray(dst, dtype=np.int64)

    in_maps, sched = _host_schedule(feature, W, b, src, dst)
    key = (sched["NT"], tuple(sched["T_LO"].tolist()), tuple(sched["T_HI"].tolist()))
    if key not in _CACHE:
        _CACHE[key] = _build(sched)
    nc = _CACHE[key]
    res = run_bass_kernel_spmd(nc, in_maps, core_ids=list(range(N_CORES)))
    return _assemble([res.results[c]["out"] for c in range(N_CORES)], sched)
